# revision 4
# baseline (speedup 1.0000x reference)
"""SVD++ prediction kernel for Trainium2 (8 NeuronCores, Bass/Tile).

Math (per batch element b with user u = x[b,0], item i = x[b,1]):
    y_sum  = sum_h Y[items_hist[u, h]]                  (H = 50)
    pred_b = mu + bu[u] + bi[i] + dot(P[u] + inv_sqrt[u] * y_sum, Q[i])

Strategy: pure data parallelism. The batch (16384) is split into 8 slices of
2048; each core runs an identical program over its slice. The small tables
(20000 rows actually addressable -- all ids < 20000) are replicated to every
core in gather-friendly layouts:

    hist16 [20000,128] int16 : items_hist rows cast to int16, padded to 256B
    P_ext  [20000,128] f32   : [P row | bu | inv_sqrt | pad]  (512B rows)
    Q_ext  [20000,128] f32   : [Q row | bi | pad]             (512B rows)
    Y      [20000, 64] f32   : as-is (256B rows)

On-device per core (all gathers are GPSIMD SWDGE dma_gather, queue 0):
 1. hist-gather: 4096 slots (each user's hist row gathered twice so that the
    row for b = 128c+16j+q lands on partitions 32*(j%4)+q AND 32*(j%4)+16+q,
    satisfying the engine quadrant rule for the fold below).
 2. fold: 8 quad-aligned 32-lane DVE copies build I16[16r+q, 400c+8h+j] =
    hist16[u_b, h] -- the wrapped int16 index tensor the gather ucode reads
    (idx t of chunk c at partition t%16, col t//16; replica groups 0/1 feed
    the rx/tx Q7 cores of queue 0).
 3. per chunk c (128 batch rows): Y-gather of 6400 rows -> [128, 50, 64] f32,
    dst[b%128, h, :] = Y[hist[u_b, h]]; DVE strided reduce over h -> y_sum.
 4. P_ext/Q_ext gathers (512B rows) deliver pu, bu, inv_sqrt, qi, bi in
    batch-major layout; DVE computes the prediction; one DMA writes
    out[128, 16] (pred of b = 128c+p at [p, c]); host untransposes.
"""
import os
import sys
import numpy as np
from contextlib import ExitStack

if "/opt/trn_rl_repo" not in sys.path:
    sys.path.insert(0, "/opt/trn_rl_repo")

import concourse.bacc as bacc
import concourse.tile as tile
import concourse.mybir as mybir
from concourse.bass_utils import run_bass_kernel_spmd

N_CORES = 8
B = 16384
BC = B // N_CORES          # per-core batch = 2048
C = BC // 128              # chunks of 128 batch rows = 16
F = 64                     # factors
H = 50                     # history length
HP = 128                   # padded hist row (int16 -> 256B)
NI = 20000                 # addressable table rows (all ids < 20000)

_PROGRAM_CACHE = {}
LAST_RESULTS = None        # side-channel for test harness (profile access)


def _build_program():
    nc = bacc.Bacc("TRN2", target_bir_lowering=False, debug=False,
                   num_devices=N_CORES)

    yt = nc.dram_tensor("Y", [NI, F], mybir.dt.float32, kind="ExternalInput")
    histT = nc.dram_tensor("hist16", [NI, HP], mybir.dt.int16, kind="ExternalInput")
    pextT = nc.dram_tensor("P_ext", [NI, 128], mybir.dt.float32, kind="ExternalInput")
    qextT = nc.dram_tensor("Q_ext", [NI, 128], mybir.dt.float32, kind="ExternalInput")
    uhT = nc.dram_tensor("uh_wrap", [128, 2 * BC // 16], mybir.dt.int16, kind="ExternalInput")
    uwT = nc.dram_tensor("u_wrap", [128, BC // 16], mybir.dt.int16, kind="ExternalInput")
    iwT = nc.dram_tensor("i_wrap", [128, BC // 16], mybir.dt.int16, kind="ExternalInput")
    muT = nc.dram_tensor("mu", [128, 1], mybir.dt.float32, kind="ExternalInput")
    outT = nc.dram_tensor("out", [128, C], mybir.dt.float32, kind="ExternalOutput")

    with tile.TileContext(nc) as tc, ExitStack() as ctx:
        pool = ctx.enter_context(tc.tile_pool(name="main", bufs=1))
        gpool = ctx.enter_context(tc.tile_pool(name="yg", bufs=3))

        uhw = pool.tile([128, 2 * BC // 16], mybir.dt.int16)
        nc.sync.dma_start(uhw[:], uhT[:])
        uw = pool.tile([128, BC // 16], mybir.dt.int16)
        nc.sync.dma_start(uw[:], uwT[:])
        iw = pool.tile([128, BC // 16], mybir.dt.int16)
        nc.sync.dma_start(iw[:], iwT[:])
        muS = pool.tile([128, 1], mybir.dt.float32)
        nc.sync.dma_start(muS[:], muT[:])

        # 1. first-level gathers
        hist_t = pool.tile([128, 2 * C, HP], mybir.dt.int16)
        nc.gpsimd.dma_gather(hist_t[:], histT[:], uhw[:], 2 * BC, 2 * BC, HP,
                             single_packet=False)
        pg = pool.tile([128, C, 128], mybir.dt.float32)
        nc.gpsimd.dma_gather(pg[:], pextT[:], uw[:], BC, BC, 128,
                             single_packet=False)
        qg = pool.tile([128, C, 128], mybir.dt.float32)
        nc.gpsimd.dma_gather(qg[:], qextT[:], iw[:], BC, BC, 128,
                             single_packet=False)

        # 2. fold -> wrapped Y-index tensor
        I16 = pool.tile([128, C * 400], mybir.dt.int16)
        nc.vector.memset(I16[:], 0)
        I16v = I16[:].rearrange("p (c h j) -> p c h j", c=C, h=H, j=8)
        hv = hist_t[:].rearrange("p (c two) e -> p c two e", two=2)
        for j in range(8):
            s = 32 * (j % 4)
            nc.vector.tensor_copy(I16v[0:32, :, :, j], hv[s:s + 32, :, j // 4, 0:H])

        # 3. Y gathers + segment reduce
        ysum = pool.tile([128, C, F], mybir.dt.float32)
        for c in range(C):
            g = gpool.tile([128, H, F], mybir.dt.float32, tag="yg")
            nc.gpsimd.dma_gather(
                g[:], yt[:], I16[:, c * 400:(c + 1) * 400], 128 * H, 128 * H, F,
                single_packet=False)
            nc.vector.reduce_sum(
                ysum[:, c, :], g[:].rearrange("p h f -> p f h"),
                axis=mybir.AxisListType.X)

        # 4. prediction
        put = pool.tile([128, F], mybir.dt.float32, tag="put")
        tmp = pool.tile([128, F], mybir.dt.float32, tag="tmp")
        sall = pool.tile([128, C], mybir.dt.float32)
        for c in range(C):
            nc.vector.tensor_scalar(
                put[:, :], ysum[:, c, :], pg[:, c, F + 1:F + 2], None,
                mybir.AluOpType.mult)
            nc.vector.tensor_add(put[:, :], put[:, :], pg[:, c, 0:F])
            nc.vector.tensor_mul(tmp[:, :], put[:, :], qg[:, c, 0:F])
            nc.vector.reduce_sum(sall[:, c:c + 1], tmp[:, :],
                                 axis=mybir.AxisListType.X)
        nc.vector.tensor_add(sall[:, :], sall[:, :], pg[:, :, F])
        nc.vector.tensor_add(sall[:, :], sall[:, :], qg[:, :, F])
        ot = pool.tile([128, C], mybir.dt.float32)
        nc.vector.tensor_scalar_add(ot[:, :], sall[:, :], muS[:, 0:1])
        nc.sync.dma_start(outT[:, :], ot[:, :])

    nc.compile()
    return nc


def _wrap16(v, n):
    # idx t read from [t%16, t//16]; replicate the 16-partition block x8
    w = np.ascontiguousarray(v.astype(np.int16).reshape(n // 16, 16).T)
    return np.tile(w, (8, 1))


def build_in_maps(inputs):
    """Host-side shard/prep: per-core input dicts for run_bass_kernel_spmd."""
    x = np.asarray(inputs["x"])
    items_hist = np.asarray(inputs["items_hist"])
    P = np.asarray(inputs["P"], np.float32)
    Q = np.asarray(inputs["Q"], np.float32)
    bu = np.asarray(inputs["bu"], np.float32)
    bi = np.asarray(inputs["bi"], np.float32)
    Y = np.asarray(inputs["Y"], np.float32)
    inv_sqrt = np.asarray(inputs["inv_sqrt"], np.float32)
    mu = np.float32(np.asarray(inputs["mu"]))

    # shared table prep (all referenced ids are < NI)
    hist16 = np.zeros((NI, HP), np.int16)
    hist16[:, :H] = items_hist[:NI].astype(np.int16)
    P_ext = np.zeros((NI, 128), np.float32)
    P_ext[:, :F] = P[:NI]
    P_ext[:, F] = bu[:NI]
    P_ext[:, F + 1] = inv_sqrt[:NI]
    Q_ext = np.zeros((NI, 128), np.float32)
    Q_ext[:, :F] = Q[:NI]
    Q_ext[:, F] = bi[:NI]
    Yc = np.ascontiguousarray(Y[:NI])
    mu_arr = np.full((128, 1), mu, np.float32)

    # hist-gather slot map (same for every core)
    bb = np.arange(BC)
    qq, jj, cc = bb % 16, (bb % 128) // 16, bb // 128
    i1_base = qq + 32 * (jj % 4) + 128 * (2 * cc + jj // 4)

    in_maps = []
    for core in range(N_CORES):
        sl = slice(core * BC, (core + 1) * BC)
        u = x[sl, 0].astype(np.int16)
        it = x[sl, 1].astype(np.int16)
        hist_slots = np.zeros(2 * BC, np.int16)
        hist_slots[i1_base] = u
        hist_slots[i1_base + 16] = u
        in_maps.append({
            "Y": Yc, "hist16": hist16, "P_ext": P_ext, "Q_ext": Q_ext,
            "uh_wrap": _wrap16(hist_slots, 2 * BC),
            "u_wrap": _wrap16(u, BC),
            "i_wrap": _wrap16(it, BC),
            "mu": mu_arr,
        })

    return in_maps


def kernel(x, items_hist, P, Q, bu, bi, Y, inv_sqrt, mu):
    global LAST_RESULTS
    if "prog" not in _PROGRAM_CACHE:
        _PROGRAM_CACHE["prog"] = _build_program()
    nc = _PROGRAM_CACHE["prog"]

    in_maps = build_in_maps(dict(x=x, items_hist=items_hist, P=P, Q=Q, bu=bu,
                                 bi=bi, Y=Y, inv_sqrt=inv_sqrt, mu=mu))
    res = run_bass_kernel_spmd(nc, in_maps, list(range(N_CORES)))
    LAST_RESULTS = res

    pred = np.empty(B, np.float32)
    for core in range(N_CORES):
        o = res.results[core]["out"]            # [128, C]; b = 128c + p
        pred[core * BC:(core + 1) * BC] = o.T.reshape(-1)
    return pred


# revision 5
# speedup vs baseline: 1.0579x; 1.0579x over previous
"""SVD++ prediction kernel for Trainium2 (8 NeuronCores, Bass/Tile).

Math (per batch element b with user u = x[b,0], item i = x[b,1]):
    y_sum  = sum_h Y[items_hist[u, h]]                  (H = 50)
    pred_b = mu + bu[u] + bi[i] + dot(P[u] + inv_sqrt[u] * y_sum, Q[i])

Strategy: pure data parallelism. The batch (16384) is split into 8 slices of
2048; each core runs an identical program over its slice. The small tables
(20000 rows actually addressable -- all ids < 20000) are replicated to every
core in gather-friendly layouts:

    hist16 [20000,128] int16 : items_hist rows cast to int16, padded to 256B
    P_ext  [20000,128] f32   : [P row | bu | inv_sqrt | pad]  (512B rows)
    Q_ext  [20000,128] f32   : [Q row | bi | pad]             (512B rows)
    Y      [20000, 64] f32   : as-is (256B rows)

On-device per core (all gathers are GPSIMD SWDGE dma_gather, queue 0):
 1. hist-gather: 4096 slots (each user's hist row gathered twice so that the
    row for b = 128c+16j+q lands on partitions 32*(j%4)+q AND 32*(j%4)+16+q,
    satisfying the engine quadrant rule for the fold below).
 2. fold: 8 quad-aligned 32-lane DVE copies build I16[16r+q, 400c+8h+j] =
    hist16[u_b, h] -- the wrapped int16 index tensor the gather ucode reads
    (idx t of chunk c at partition t%16, col t//16; replica groups 0/1 feed
    the rx/tx Q7 cores of queue 0).
 3. per chunk c (128 batch rows): Y-gather of 6400 rows -> [128, 50, 64] f32,
    dst[b%128, h, :] = Y[hist[u_b, h]]; DVE strided reduce over h -> y_sum.
 4. P_ext/Q_ext gathers (512B rows) deliver pu, bu, inv_sqrt, qi, bi in
    batch-major layout; DVE computes the prediction; one DMA writes
    out[128, 16] (pred of b = 128c+p at [p, c]); host untransposes.
"""
import os
import sys
import numpy as np
from contextlib import ExitStack

if "/opt/trn_rl_repo" not in sys.path:
    sys.path.insert(0, "/opt/trn_rl_repo")

import concourse.bacc as bacc
import concourse.tile as tile
import concourse.mybir as mybir
from concourse.bass_utils import run_bass_kernel_spmd

N_CORES = 8
B = 16384
BC = B // N_CORES          # per-core batch = 2048
C = BC // 128              # chunks of 128 batch rows = 16
F = 64                     # factors
H = 50                     # history length
HP = 128                   # padded hist row (int16 -> 256B)
NI = 20000                 # addressable table rows (all ids < 20000)

_PROGRAM_CACHE = {}
LAST_RESULTS = None        # side-channel for test harness (profile access)


def _build_program(reps=1, sim_safe=False):
    nc = bacc.Bacc("TRN2", target_bir_lowering=False, debug=False,
                   num_devices=N_CORES)

    yt = nc.dram_tensor("Y", [NI, F], mybir.dt.float32, kind="ExternalInput")
    histT = nc.dram_tensor("hist16", [NI, HP], mybir.dt.int16, kind="ExternalInput")
    pextT = nc.dram_tensor("P_ext", [NI, 128], mybir.dt.float32, kind="ExternalInput")
    qextT = nc.dram_tensor("Q_ext", [NI, 128], mybir.dt.float32, kind="ExternalInput")
    uhT = nc.dram_tensor("uh_wrap", [128, 2 * BC // 16], mybir.dt.int16, kind="ExternalInput")
    uwT = nc.dram_tensor("u_wrap", [128, BC // 16], mybir.dt.int16, kind="ExternalInput")
    iwT = nc.dram_tensor("i_wrap", [128, BC // 16], mybir.dt.int16, kind="ExternalInput")
    muT = nc.dram_tensor("mu", [128, 1], mybir.dt.float32, kind="ExternalInput")
    outT = nc.dram_tensor("out", [128, C], mybir.dt.float32, kind="ExternalOutput")

    with tile.TileContext(nc) as tc, ExitStack() as ctx:
        pool = ctx.enter_context(tc.tile_pool(name="main", bufs=1))
        gpool = ctx.enter_context(tc.tile_pool(name="yg", bufs=3))

        uhw = pool.tile([128, 2 * BC // 16], mybir.dt.int16)
        nc.sync.dma_start(uhw[:], uhT[:])
        uw = pool.tile([128, BC // 16], mybir.dt.int16)
        nc.sync.dma_start(uw[:], uwT[:])
        iw = pool.tile([128, BC // 16], mybir.dt.int16)
        nc.sync.dma_start(iw[:], iwT[:])
        muS = pool.tile([128, 1], mybir.dt.float32)
        nc.sync.dma_start(muS[:], muT[:])

        for _rep in range(reps):
            # 1. hist gather + fold -> wrapped Y-index tensor
            hist_t = pool.tile([128, 2 * C, HP], mybir.dt.int16, tag="hist_t")
            nc.gpsimd.dma_gather(hist_t[:], histT[:], uhw[:], 2 * BC, 2 * BC, HP,
                                 single_packet=False)
            I16 = pool.tile([128, C * 400], mybir.dt.int16, tag="I16")
            if sim_safe:
                nc.vector.memset(I16[:], 0)
            I16v = I16[:].rearrange("p (c h j) -> p c h j", c=C, h=H, j=8)
            hv = hist_t[:].rearrange("p (c two) e -> p c two e", two=2)
            for j in range(8):
                s = 32 * (j % 4)
                nc.vector.tensor_copy(I16v[0:32, :, :, j],
                                      hv[s:s + 32, :, j // 4, 0:H])

            # 2. Y gathers + segment reduce
            ysum = pool.tile([128, C, F], mybir.dt.float32, tag="ysum")
            for c in range(C):
                g = gpool.tile([128, H, F], mybir.dt.float32, tag="yg")
                nc.gpsimd.dma_gather(
                    g[:], yt[:], I16[:, c * 400:(c + 1) * 400], 128 * H, 128 * H, F,
                    single_packet=False)
                nc.vector.reduce_sum(
                    ysum[:, c, :], g[:].rearrange("p h f -> p f h"),
                    axis=mybir.AxisListType.X)

            # 3. P/Q gathers (issued after the Y stream is queued)
            pg = pool.tile([128, C, 128], mybir.dt.float32, tag="pg")
            nc.gpsimd.dma_gather(pg[:], pextT[:], uw[:], BC, BC, 128,
                                 single_packet=False)
            qg = pool.tile([128, C, 128], mybir.dt.float32, tag="qg")
            nc.gpsimd.dma_gather(qg[:], qextT[:], iw[:], BC, BC, 128,
                                 single_packet=False)

            # 4. prediction
            put = pool.tile([128, F], mybir.dt.float32, tag="put")
            tmp = pool.tile([128, F], mybir.dt.float32, tag="tmp")
            sall = pool.tile([128, C], mybir.dt.float32, tag="sall")
            for c in range(C):
                nc.vector.tensor_scalar(
                    put[:, :], ysum[:, c, :], pg[:, c, F + 1:F + 2], None,
                    mybir.AluOpType.mult)
                nc.vector.tensor_add(put[:, :], put[:, :], pg[:, c, 0:F])
                nc.vector.tensor_mul(tmp[:, :], put[:, :], qg[:, c, 0:F])
                nc.vector.reduce_sum(sall[:, c:c + 1], tmp[:, :],
                                     axis=mybir.AxisListType.X)
            nc.vector.tensor_add(sall[:, :], sall[:, :], pg[:, :, F])
            nc.vector.tensor_add(sall[:, :], sall[:, :], qg[:, :, F])
            ot = pool.tile([128, C], mybir.dt.float32, tag="ot")
            nc.vector.tensor_scalar_add(ot[:, :], sall[:, :], muS[:, 0:1])
            nc.sync.dma_start(outT[:, :], ot[:, :])

    nc.compile()
    return nc


def _wrap16(v, n):
    # idx t read from [t%16, t//16]; replicate the 16-partition block x8
    w = np.ascontiguousarray(v.astype(np.int16).reshape(n // 16, 16).T)
    return np.tile(w, (8, 1))


def build_in_maps(inputs):
    """Host-side shard/prep: per-core input dicts for run_bass_kernel_spmd."""
    x = np.asarray(inputs["x"])
    items_hist = np.asarray(inputs["items_hist"])
    P = np.asarray(inputs["P"], np.float32)
    Q = np.asarray(inputs["Q"], np.float32)
    bu = np.asarray(inputs["bu"], np.float32)
    bi = np.asarray(inputs["bi"], np.float32)
    Y = np.asarray(inputs["Y"], np.float32)
    inv_sqrt = np.asarray(inputs["inv_sqrt"], np.float32)
    mu = np.float32(np.asarray(inputs["mu"]))

    # shared table prep (all referenced ids are < NI)
    hist16 = np.zeros((NI, HP), np.int16)
    hist16[:, :H] = items_hist[:NI].astype(np.int16)
    P_ext = np.zeros((NI, 128), np.float32)
    P_ext[:, :F] = P[:NI]
    P_ext[:, F] = bu[:NI]
    P_ext[:, F + 1] = inv_sqrt[:NI]
    Q_ext = np.zeros((NI, 128), np.float32)
    Q_ext[:, :F] = Q[:NI]
    Q_ext[:, F] = bi[:NI]
    Yc = np.ascontiguousarray(Y[:NI])
    mu_arr = np.full((128, 1), mu, np.float32)

    # hist-gather slot map (same for every core)
    bb = np.arange(BC)
    qq, jj, cc = bb % 16, (bb % 128) // 16, bb // 128
    i1_base = qq + 32 * (jj % 4) + 128 * (2 * cc + jj // 4)

    in_maps = []
    for core in range(N_CORES):
        sl = slice(core * BC, (core + 1) * BC)
        u = x[sl, 0].astype(np.int16)
        it = x[sl, 1].astype(np.int16)
        hist_slots = np.zeros(2 * BC, np.int16)
        hist_slots[i1_base] = u
        hist_slots[i1_base + 16] = u
        in_maps.append({
            "Y": Yc, "hist16": hist16, "P_ext": P_ext, "Q_ext": Q_ext,
            "uh_wrap": _wrap16(hist_slots, 2 * BC),
            "u_wrap": _wrap16(u, BC),
            "i_wrap": _wrap16(it, BC),
            "mu": mu_arr,
        })

    return in_maps


def kernel(x, items_hist, P, Q, bu, bi, Y, inv_sqrt, mu):
    global LAST_RESULTS
    if "prog" not in _PROGRAM_CACHE:
        _PROGRAM_CACHE["prog"] = _build_program()
    nc = _PROGRAM_CACHE["prog"]

    in_maps = build_in_maps(dict(x=x, items_hist=items_hist, P=P, Q=Q, bu=bu,
                                 bi=bi, Y=Y, inv_sqrt=inv_sqrt, mu=mu))
    res = run_bass_kernel_spmd(nc, in_maps, list(range(N_CORES)))
    LAST_RESULTS = res

    pred = np.empty(B, np.float32)
    for core in range(N_CORES):
        o = res.results[core]["out"]            # [128, C]; b = 128c + p
        pred[core * BC:(core + 1) * BC] = o.T.reshape(-1)
    return pred
